# revision 1
# baseline (speedup 1.0000x reference)
"""AttnBlock1d Trainium2 kernel: 8-core SPMD, zero-collective sharding.

Sharding: core i handles (batch b = i//2, N-half = i%2). The input x[b] is
host-rolled along N so every core's query half sits at columns 0:1024 —
groupnorm stats, k/v (pointwise in N) and softmax are permutation-invariant
along N, so rolling commutes with everything except the q slice, which is
exactly the point.

Per-core pipeline (N=2048 keys, NQ=1024 queries, C=512, H=8 heads):
  groupnorm(x) -> h_ (bf16)
  q = qw@h_[:, :1024]+qb, k = kw@h_+kb (bf16)
  vT[n, c] = h_^T @ v_w^T (bf16, per-head 65-col blocks with a ones column)
  per head: scoresT[nk, nq] = k_h^T q_h (PSUM f32, head pairs row-packed on
  the PE array), exp via ScalarE (scale folded), out_u = vT_aug^T @ exp
  (M=65: row 64 accumulates the softmax denominator D), divide by D
  (DVE reciprocal + DMA partition-broadcast), + v_b
  proj + proj_b + residual x -> out[512, 1024]
"""

import os
import sys

import numpy as np

if "/opt/trn_rl_repo" not in sys.path:
    sys.path.insert(0, "/opt/trn_rl_repo")

import ml_dtypes

import concourse.bacc as bacc
import concourse.tile as tile
from concourse import mybir
from concourse.bass_utils import run_bass_kernel_spmd

F32 = mybir.dt.float32
BF16 = mybir.dt.bfloat16
AF = mybir.ActivationFunctionType
ALU = mybir.AluOpType

C = 512
N = 2048
NQ = 1024
H = 8
HC = 64
G = 32
EPS = 1e-6
SCALE = 1.0 / np.sqrt(C)

TRACE = False
LAST_RESULT = None


DEBUG_TAPS = False


def build_bacc():
    nc = bacc.Bacc()
    dbg = {}
    if DEBUG_TAPS:
        dbg["h0"] = nc.declare_dram_parameter("dbg_h0", [128, N], BF16, isOutput=True)
        dbg["mvg"] = nc.declare_dram_parameter("dbg_mvg", [G, 2], F32, isOutput=True)
        dbg["varg"] = nc.declare_dram_parameter("dbg_varg", [G, 1], F32, isOutput=True)
        dbg["q0"] = nc.declare_dram_parameter("dbg_q0", [128, NQ], BF16, isOutput=True)
        dbg["k0"] = nc.declare_dram_parameter("dbg_k0", [128, N], BF16, isOutput=True)
        dbg["vt0"] = nc.declare_dram_parameter("dbg_vt0", [128, H * 65], BF16, isOutput=True)
        dbg["exp0"] = nc.declare_dram_parameter("dbg_exp0", [128, NQ], BF16, isOutput=True)
        dbg["op0"] = nc.declare_dram_parameter("dbg_op0", [65, 512], F32, isOutput=True)
        dbg["s2"] = nc.declare_dram_parameter("dbg_s2", [128, 2], F32, isOutput=True)
        dbg["gps"] = nc.declare_dram_parameter("dbg_gps", [G, 2], F32, isOutput=True)
        dbg["attn0"] = nc.declare_dram_parameter("dbg_attn0", [128, NQ], BF16, isOutput=True)

    x_d = nc.declare_dram_parameter("x", [C, N], F32, isOutput=False)
    qwt_d = nc.declare_dram_parameter("qwt", [C, C], BF16, isOutput=False)
    kwt_d = nc.declare_dram_parameter("kwt", [C, C], BF16, isOutput=False)
    vwt_d = nc.declare_dram_parameter("vwt", [C, C], BF16, isOutput=False)
    pwt_d = nc.declare_dram_parameter("pwt", [C, C], BF16, isOutput=False)
    qb_d = nc.declare_dram_parameter("qb", [C, 1], F32, isOutput=False)
    kb_d = nc.declare_dram_parameter("kb", [C, 1], F32, isOutput=False)
    pb_d = nc.declare_dram_parameter("pb", [C, 1], F32, isOutput=False)
    vbh_d = nc.declare_dram_parameter("vbh", [HC, H], F32, isOutput=False)
    gam_d = nc.declare_dram_parameter("gam", [C, 1], F32, isOutput=False)
    bet_d = nc.declare_dram_parameter("bet", [C, 1], F32, isOutput=False)
    gmap_d = nc.declare_dram_parameter("gmap", [C, G], F32, isOutput=False)
    gmapt_d = nc.declare_dram_parameter("gmapt", [G, C], F32, isOutput=False)
    out_d = nc.declare_dram_parameter("out", [C, NQ], F32, isOutput=True)

    from contextlib import ExitStack

    with tile.TileContext(nc) as tc, ExitStack() as es:
        const = es.enter_context(tc.tile_pool(name="const", bufs=1))
        data = es.enter_context(tc.tile_pool(name="data", bufs=1))
        work = es.enter_context(tc.tile_pool(name="work", bufs=2))
        expp = es.enter_context(tc.tile_pool(name="expp", bufs=24))
        osbp = es.enter_context(tc.tile_pool(name="osbp", bufs=3))
        psA = es.enter_context(tc.tile_pool(name="psA", bufs=4, space="PSUM"))
        psB = es.enter_context(tc.tile_pool(name="psB", bufs=2, space="PSUM"))
        dpool = es.enter_context(tc.tile_pool(name="dpool", bufs=4, space="DRAM"))

        xs = []
        xdma = [nc.sync, nc.scalar, nc.sync, nc.scalar]
        for t in range(4):
            xt = data.tile([128, N], F32, tag=f"x{t}")
            eng = xdma[t % len(xdma)]
            eng.dma_start(out=xt[:, 0:1024], in_=x_d[t * 128:(t + 1) * 128, 0:1024])
            eng.dma_start(out=xt[:, 1024:2048],
                          in_=x_d[t * 128:(t + 1) * 128, 1024:2048])
            xs.append(xt)

        # ---- constant loads ----
        def load4(dram, shape, dt, tagp):
            ts = []
            for t in range(4):
                s = const.tile(shape, dt, tag=f"{tagp}{t}")
                nc.gpsimd.dma_start(out=s, in_=dram[t * 128:(t + 1) * 128, :])
                ts.append(s)
            return ts

        # loads ordered by first use: gn consts, then v/q/k weights, then rest
        gmap = load4(gmap_d, [128, G], F32, "gmap")
        gam = load4(gam_d, [128, 1], F32, "gam")
        bet = load4(bet_d, [128, 1], F32, "bet")
        gmapt = const.tile([G, C], F32, tag="gmapt")
        nc.gpsimd.dma_start(out=gmapt, in_=gmapt_d[:, :])
        eps32 = const.tile([G, 1], F32, tag="eps32")
        nc.vector.memset(eps32, EPS)
        vwt = load4(vwt_d, [128, C], BF16, "vwt")
        qwt = load4(qwt_d, [128, C], BF16, "qwt")
        kwt = load4(kwt_d, [128, C], BF16, "kwt")
        qb = load4(qb_d, [128, 1], F32, "qb")
        kb = load4(kb_d, [128, 1], F32, "kb")
        vbh = const.tile([HC, H], F32, tag="vbh")
        nc.gpsimd.dma_start(out=vbh, in_=vbh_d[:, :])
        pwt = load4(pwt_d, [128, C], BF16, "pwt")
        pb = load4(pb_d, [128, 1], F32, "pb")

        # ---- groupnorm stats ----
        stats2s = []
        for t in range(4):
            st = work.tile([128, 4, 6], F32, tag="bnst")
            for sg in range(4):
                nc.vector.bn_stats(out=st[:, sg, :], in_=xs[t][:, sg * 512:(sg + 1) * 512])
            mv = work.tile([128, 2], F32, tag="bnmv")
            nc.vector.bn_aggr(out=mv, in_=st)
            s2 = work.tile([128, 2], F32, tag=f"s2_{t}")
            nc.vector.tensor_copy(out=s2[:, 0:1], in_=mv[:, 0:1])
            nc.vector.tensor_mul(out=s2[:, 1:2], in0=mv[:, 0:1], in1=mv[:, 0:1])
            nc.vector.tensor_add(out=s2[:, 1:2], in0=s2[:, 1:2], in1=mv[:, 1:2])
            if DEBUG_TAPS and t == 0:
                nc.sync.dma_start(out=dbg["s2"][:, :], in_=s2)
            stats2s.append(s2)

        gps = psA.tile([128, 512], F32, tag="mm")
        for t in range(4):
            nc.tensor.matmul(out=gps[0:G, 0:2], lhsT=gmap[t], rhs=stats2s[t],
                             start=(t == 0), stop=(t == 3))
        if DEBUG_TAPS:
            gpsc = work.tile([G, 2], F32, tag="gpsc")
            nc.vector.tensor_copy(out=gpsc, in_=gps[0:G, 0:2])
            nc.sync.dma_start(out=dbg["gps"][:, :], in_=gpsc)
        mvg = work.tile([G, 2], F32, tag="mvg")
        nc.vector.tensor_scalar(out=mvg, in0=gps[0:G, 0:2], scalar1=1.0 / 16,
                                scalar2=None, op0=ALU.mult)
        varg = work.tile([G, 1], F32, tag="varg")
        nc.vector.tensor_mul(out=varg, in0=mvg[:, 0:1], in1=mvg[:, 0:1])
        nc.vector.tensor_tensor(out=varg, in0=mvg[:, 1:2], in1=varg, op=ALU.subtract)
        sd = work.tile([G, 1], F32, tag="sd")
        nc.scalar.activation(out=sd, in_=varg, func=AF.Sqrt, bias=eps32)
        if DEBUG_TAPS:
            nc.sync.dma_start(out=dbg["varg"][:, :], in_=varg)
            nc.sync.dma_start(out=dbg["mvg"][:, :], in_=mvg)
        rsg = work.tile([G, 1], F32, tag="rsg")
        nc.vector.reciprocal(out=rsg, in_=sd)
        gvals = work.tile([G, 2], F32, tag="gvals")
        nc.vector.tensor_copy(out=gvals[:, 0:1], in_=rsg)
        nc.vector.tensor_copy(out=gvals[:, 1:2], in_=mvg[:, 0:1])

        hs = []
        for t in range(4):
            bc = psA.tile([128, 512], F32, tag="mm")
            nc.tensor.matmul(out=bc[:, 0:2], lhsT=gmapt[:, t * 128:(t + 1) * 128],
                             rhs=gvals, start=True, stop=True)
            a_t = work.tile([128, 1], F32, tag="a_t")
            nc.vector.tensor_mul(out=a_t, in0=bc[:, 0:1], in1=gam[t])
            b_t = work.tile([128, 1], F32, tag="b_t")
            nc.vector.tensor_mul(out=b_t, in0=bc[:, 1:2], in1=a_t)
            nc.vector.tensor_tensor(out=b_t, in0=bet[t], in1=b_t, op=ALU.subtract)
            ht = data.tile([128, N], BF16, tag=f"h{t}")
            nc.vector.tensor_scalar(out=ht, in0=xs[t], scalar1=a_t, scalar2=b_t,
                                    op0=ALU.mult, op1=ALU.add)
            if DEBUG_TAPS and t == 0:
                nc.sync.dma_start(out=dbg["h0"][:, :], in_=ht)
            hs.append(ht)

        # ---- attention core with q/k projections pipelined per head-pair ----
        attns = []
        for ct in range(4):
            at = data.tile([128, NQ], BF16, tag=f"attn{ct}")
            attns.append(at)

        vts = {}

        def emit_vt(mt):
            vt = data.tile([128, H * 65], BF16, tag=f"vt{mt}", name=f"vt{mt}")
            nc.vector.memset(vt, 1.0)
            ps = psA.tile([128, 512], F32, tag="mm", name=f"vtps{mt}")
            for kt in range(4):
                nc.tensor.matmul(out=ps, lhsT=hs[kt][:, mt * 128:(mt + 1) * 128],
                                 rhs=vwt[kt][:, 0:512], start=(kt == 0), stop=(kt == 3))
            nc.vector.tensor_copy(
                out=vt.rearrange("p (h w) -> p h w", h=H)[:, :, 0:HC],
                in_=ps.rearrange("p (h w) -> p h w", h=H),
            )
            if DEBUG_TAPS and mt == 0:
                nc.sync.dma_start(out=dbg["vt0"][:, :], in_=vt)
            vts[mt] = vt

        def emit_qk(ct):
            qt = data.tile([128, NQ], BF16, tag=f"q{ct}", name=f"q{ct}")
            for n in range(2):
                ps = psA.tile([128, 512], F32, tag="mm", name=f"qps{ct}_{n}")
                for kt in range(4):
                    nc.tensor.matmul(out=ps, lhsT=qwt[kt][:, ct * 128:(ct + 1) * 128],
                                     rhs=hs[kt][:, n * 512:(n + 1) * 512],
                                     start=(kt == 0), stop=(kt == 3))
                nc.vector.tensor_scalar(out=qt[:, n * 512:(n + 1) * 512], in0=ps,
                                        scalar1=qb[ct], scalar2=None, op0=ALU.add)
            ktile = data.tile([128, N], BF16, tag=f"k{ct}", name=f"k{ct}")
            for n in range(4):
                ps = psA.tile([128, 512], F32, tag="mm", name=f"kps{ct}_{n}")
                for kt in range(4):
                    nc.tensor.matmul(out=ps, lhsT=kwt[kt][:, ct * 128:(ct + 1) * 128],
                                     rhs=hs[kt][:, n * 512:(n + 1) * 512],
                                     start=(kt == 0), stop=(kt == 3))
                nc.vector.tensor_scalar(out=ktile[:, n * 512:(n + 1) * 512], in0=ps,
                                        scalar1=kb[ct], scalar2=None, op0=ALU.add)
            return qt, ktile

        qks = {0: emit_qk(0)}
        early_pss = []

        for ct in range(4):
            qt, ktile = qks[ct]
            if DEBUG_TAPS and ct == 0:
                nc.sync.dma_start(out=dbg["q0"][:, :], in_=qt)
                nc.sync.dma_start(out=dbg["k0"][:, :], in_=ktile)

            oc = work.tile([65, 4 * 512], F32, tag="oc")
            recd2 = dpool.tile([1, 4 * 512], F32, tag="recd2")
            for n in range(2):
                opsn = [psA.tile([128, 512], F32, tag="mm", name=f"op_{ct}_{n}_{hp}")
                        for hp in range(2)]
                for mt in range(16):
                    sc = psB.tile([128, NQ], F32, tag="sc", name=f"sc_{ct}_{mt}_{n}")
                    for hp in range(2):
                        hb = hp * 64
                        nc.tensor.matmul(
                            out=sc[:, hp * 512:(hp + 1) * 512],
                            lhsT=ktile[hb:hb + 64, mt * 128:(mt + 1) * 128],
                            rhs=qt[hb:hb + 64, n * 512:(n + 1) * 512],
                            start=True, stop=True)
                    et = expp.tile([128, NQ], BF16, tag="exp", name=f"et_{ct}_{mt}_{n}")
                    nc.scalar.activation(out=et, in_=sc, func=AF.Exp, scale=float(SCALE))
                    if DEBUG_TAPS and ct == 0 and mt == 0 and n == 0:
                        nc.sync.dma_start(out=dbg["exp0"][:, 0:512], in_=et[:, 0:512])
                        nc.sync.dma_start(out=dbg["exp0"][:, 512:1024], in_=et[:, 512:1024])
                    if mt not in vts:
                        emit_vt(mt)
                    for hp in range(2):
                        h = 2 * ct + hp
                        nc.tensor.matmul(
                            out=opsn[hp][0:65, :],
                            lhsT=vts[mt][:, 65 * h:65 * h + 65],
                            rhs=et[:, hp * 512:(hp + 1) * 512],
                            start=(mt == 0), stop=(mt == 15))
                # prefetch next pair's q/k BEFORE this pair's division ops so
                # the in-order DVE queue isn't blocked by the recip chain waits
                if n == 0 and ct + 1 < 4:
                    qks[ct + 1] = emit_qk(ct + 1)

                # drain this n-pass: copy psum out, kick off recip chain half
                for hp in range(2):
                    nc.vector.tensor_copy(
                        out=oc[:, (hp * 2 + n) * 512:(hp * 2 + n + 1) * 512],
                        in_=opsn[hp][0:65, 0:512])
                rct = work.tile([128, 8], F32, tag="rct", name=f"rct{ct}_{n}")
                recd1 = dpool.tile([1, 2 * 512], F32, tag="recd1",
                                   name=f"recd1_{ct}_{n}")
                r1v = recd1.rearrange("o (r c p) -> (o r) c p", r=2, p=128)
                r2v = recd2.rearrange("o (r c p) -> (o r) c p", r=4, p=128)
                for hh in range(2):
                    nc.sync.dma_start(
                        out=recd1[0:1, hh * 512:(hh + 1) * 512],
                        in_=oc[64:65, (hh * 2 + n) * 512:(hh * 2 + n + 1) * 512])
                for hh in range(2):
                    nc.sync.dma_start(
                        out=rct[:, hh * 4:(hh + 1) * 4],
                        in_=r1v[hh].rearrange("c p -> p c"))
                nc.vector.reciprocal(out=rct, in_=rct)
                for hh in range(2):
                    nc.sync.dma_start(
                        out=r2v[hh * 2 + n].rearrange("c p -> p c"),
                        in_=rct[:, hh * 4:(hh + 1) * 4])
                for hp in range(2):
                    h = 2 * ct + hp
                    recbh = work.tile([64, 512], F32, tag="recb",
                                      name=f"recb{ct}_{n}_{hp}")
                    nc.gpsimd.dma_start(
                        out=recbh,
                        in_=recd2[0:1, (hp * 2 + n) * 512:(hp * 2 + n + 1) * 512]
                        .to_broadcast([64, 512]))
                    tmph = work.tile([64, 512], BF16, tag="tmp64",
                                     name=f"tmp{ct}_{n}_{hp}")
                    nc.vector.tensor_mul(
                        out=tmph,
                        in0=oc[0:64, (hp * 2 + n) * 512:(hp * 2 + n + 1) * 512],
                        in1=recbh)
                    nc.vector.tensor_scalar(out=tmph, in0=tmph,
                                            scalar1=vbh[:, h:h + 1],
                                            scalar2=None, op0=ALU.add)
                    nc.sync.dma_start(
                        out=attns[ct][hp * 64:hp * 64 + 64,
                                      n * 512:(n + 1) * 512],
                        in_=tmph)
            if DEBUG_TAPS and ct == 0:
                nc.sync.dma_start(out=dbg["op0"][:, :], in_=oc[0:65, 0:512])

            if DEBUG_TAPS and ct == 0:
                nc.sync.dma_start(out=dbg["attn0"][:, :], in_=attns[0])

        # ---- proj + bias + residual ----
        def proj_finish(pps, mo, n):
            osb = osbp.tile([128, 512], F32, tag="osb", name=f"osb{mo}_{n}")
            nc.vector.tensor_scalar(out=osb, in0=pps, scalar1=pb[mo],
                                    scalar2=None, op0=ALU.add)
            nc.vector.tensor_add(out=osb, in0=osb,
                                 in1=xs[mo][:, n * 512:(n + 1) * 512])
            nc.sync.dma_start(
                out=out_d[mo * 128:(mo + 1) * 128, n * 512:(n + 1) * 512],
                in_=osb)

        for pps, mo, n in early_pss:
            nc.tensor.matmul(out=pps, lhsT=pwt[3][:, mo * 128:(mo + 1) * 128],
                             rhs=attns[3][:, n * 512:(n + 1) * 512],
                             start=False, stop=True)
            proj_finish(pps, mo, n)
        rest = [(mo, n) for mo in (0, 1, 2, 3) for n in (0, 1)]
        for j, (mo, n) in enumerate(rest):
            pool, tag = (psB, "sc") if j % 2 == 0 else (psA, "mm")
            pps = pool.tile([128, 512], F32, tag=tag, name=f"pps{mo}_{n}")
            for kt in range(4):
                nc.tensor.matmul(out=pps, lhsT=pwt[kt][:, mo * 128:(mo + 1) * 128],
                                 rhs=attns[kt][:, n * 512:(n + 1) * 512],
                                 start=(kt == 0), stop=(kt == 3))
            proj_finish(pps, mo, n)

    nc.compile()
    return nc


_NC_CACHE = None


def _get_nc():
    global _NC_CACHE
    if _NC_CACHE is None:
        _NC_CACHE = build_bacc()
    return _NC_CACHE


def kernel(x, gn_gamma, gn_beta, q_w, q_b, k_w, k_b, v_w, v_b, proj_w, proj_b):
    global LAST_RESULT
    x = np.asarray(x, np.float32)
    B = x.shape[0]
    bf = ml_dtypes.bfloat16

    gmap = np.zeros((C, G), np.float32)
    gmap[np.arange(C), np.arange(C) // 16] = 1.0

    shared = {
        "qwt": np.ascontiguousarray(np.asarray(q_w, np.float32).T.astype(bf)),
        "kwt": np.ascontiguousarray(np.asarray(k_w, np.float32).T.astype(bf)),
        "vwt": np.ascontiguousarray(np.asarray(v_w, np.float32).T.astype(bf)),
        "pwt": np.ascontiguousarray(np.asarray(proj_w, np.float32).T.astype(bf)),
        "qb": np.asarray(q_b, np.float32).reshape(C, 1),
        "kb": np.asarray(k_b, np.float32).reshape(C, 1),
        "pb": np.asarray(proj_b, np.float32).reshape(C, 1),
        "vbh": np.ascontiguousarray(np.asarray(v_b, np.float32).reshape(H, HC).T),
        "gam": np.asarray(gn_gamma, np.float32).reshape(C, 1),
        "bet": np.asarray(gn_beta, np.float32).reshape(C, 1),
        "gmap": gmap,
        "gmapt": np.ascontiguousarray(gmap.T),
    }

    in_maps = []
    for i in range(8):
        b, half = i // 2, i % 2
        xb = np.ascontiguousarray(np.roll(x[b], -half * NQ, axis=1))
        in_maps.append({"x": xb, **shared})

    nc = _get_nc()
    res = run_bass_kernel_spmd(nc, in_maps, core_ids=list(range(8)), trace=TRACE)
    LAST_RESULT = res

    out = np.empty((B, C, N), np.float32)
    for i in range(8):
        b, half = i // 2, i % 2
        out[b][:, half * NQ:(half + 1) * NQ] = res.results[i]["out"]
    return out



# revision 16
# speedup vs baseline: 1.2845x; 1.2845x over previous
"""AttnBlock1d Trainium2 kernel: 8-core SPMD, zero-collective sharding.

Sharding: core i handles (batch b = i//2, N-half = i%2). The input x[b] is
host-rolled along N so every core's query half sits at columns 0:1024 —
groupnorm stats, k/v (pointwise in N) and softmax are permutation-invariant
along N, so rolling commutes with everything except the q slice.

v2 design (ACT-paced pipeline; the Exp stream on the scalar engine is the
roofline at ~1.2us per [128,1024] tile):
  - k-bias dropped entirely (its score contribution is constant per query
    column, which softmax cancels exactly).
  - v-bias folded into the proj bias on host: pb2 = proj_b + proj_w @ v_b
    (exact, since softmax weights sum to 1).
  - Per-x-tile pipelined groupnorm (stats -> group chain -> h).
  - Steady state slot (ct=head-pair, n=query 512-block, mt=key 128-block):
    row-packed score matmul pair -> one Exp -> attnV M=65 accumulation pair
    (ones column produces the softmax denominator in psum row 64).
  - Division: reciprocal(D) on DVE from PSUM row 64, partition_broadcast on
    GpSimd (attn library), one tensor_tensor mult PSUM -> bf16 attn rows.
    Odd heads bounce through a [64,512] tmp + SBUF->SBUF DMA row shift.
  - proj: row-packed head-pair matmuls split into stage A (head pairs 0,1 —
    pipelined into the ct2/ct3 passes) + stage B (pairs 2,3 + bias + residual
    in the tail); PE is kept continuously busy with qk/vt/proj filler tasks
    so the tensor engine holds its warm 2.4 GHz p-state.
"""

import os
import sys

import numpy as np

if "/opt/trn_rl_repo" not in sys.path:
    sys.path.insert(0, "/opt/trn_rl_repo")

import ml_dtypes

import concourse.bacc as bacc
import concourse.tile as tile
from concourse import mybir
from concourse.bass_utils import run_bass_kernel_spmd

F32 = mybir.dt.float32
BF16 = mybir.dt.bfloat16
AF = mybir.ActivationFunctionType
ALU = mybir.AluOpType

C = 512
N = 2048
NQ = 1024
H = 8
HC = 64
G = 32
EPS = 1e-6
SCALE = 1.0 / np.sqrt(C)

TRACE = False
LAST_RESULT = None

import os as _os
DBG_NO_BOUNCE = _os.environ.get("DBG_NO_BOUNCE", "0") == "1"
DBG_NO_SHIFT = _os.environ.get("DBG_NO_SHIFT", "0") == "1"
DBG_NO_GPSH = _os.environ.get("DBG_NO_GPSH", "0") == "1"
DBG_NO_GPSMEMSET = _os.environ.get("DBG_NO_GPSMEMSET", "0") == "1"
DBG_NO_PROJPAIR = _os.environ.get("DBG_NO_PROJPAIR", "0") == "1"
DBG_STAGE = int(_os.environ.get("DBG_STAGE", "0"))  # 0=full 1=qkv 2=+scores 3=+av



def build_bacc():
    nc = bacc.Bacc()

    x_d = nc.declare_dram_parameter("x", [C, N], F32, isOutput=False)
    qwt_d = nc.declare_dram_parameter("qwt", [C, C], BF16, isOutput=False)
    kwt_d = nc.declare_dram_parameter("kwt", [C, C], BF16, isOutput=False)
    vwt_d = nc.declare_dram_parameter("vwt", [C, C], BF16, isOutput=False)
    pwt_d = nc.declare_dram_parameter("pwt", [C, C], BF16, isOutput=False)
    qb_d = nc.declare_dram_parameter("qb", [C, 1], F32, isOutput=False)
    pb2_d = nc.declare_dram_parameter("pb2", [C, 1], F32, isOutput=False)
    gam_d = nc.declare_dram_parameter("gam", [C, 1], F32, isOutput=False)
    bet_d = nc.declare_dram_parameter("bet", [C, 1], F32, isOutput=False)
    gmap_d = nc.declare_dram_parameter("gmap", [C, G], F32, isOutput=False)
    gmapt_d = nc.declare_dram_parameter("gmapt", [G, C], F32, isOutput=False)
    out_d = nc.declare_dram_parameter("out", [C, NQ], F32, isOutput=True)

    from contextlib import ExitStack

    with tile.TileContext(nc) as tc, ExitStack() as es:
        const = es.enter_context(tc.tile_pool(name="const", bufs=1))
        data = es.enter_context(tc.tile_pool(name="data", bufs=1))
        work = es.enter_context(tc.tile_pool(name="work", bufs=6))
        etp = es.enter_context(tc.tile_pool(name="etp", bufs=6))
        recp = es.enter_context(tc.tile_pool(name="recp", bufs=4))
        tmpp = es.enter_context(tc.tile_pool(name="tmpp", bufs=3))
        osbp = es.enter_context(tc.tile_pool(name="osbp", bufs=3))
        psSC = es.enter_context(tc.tile_pool(name="psSC", bufs=2, space="PSUM"))
        psAV = es.enter_context(tc.tile_pool(name="psAV", bufs=2, space="PSUM"))
        psGen = es.enter_context(tc.tile_pool(name="psGen", bufs=2, space="PSUM"))
        dpool = es.enter_context(tc.tile_pool(name="dpool", bufs=4, space="DRAM"))


        # ---- input x: 4 tiles x 2 halves spread over 3 DMA queues ----
        xs = [data.tile([128, N], F32, tag=f"x{t}", name=f"x{t}") for t in range(4)]
        for t, cols, eng in [
            (0, (0, 1024), nc.sync), (0, (1024, 2048), nc.scalar),
            (3, (0, 1024), nc.gpsimd), (3, (1024, 2048), nc.gpsimd),
            (1, (0, 1024), nc.sync), (1, (1024, 2048), nc.scalar),
            (2, (0, 1024), nc.sync), (2, (1024, 2048), nc.scalar),
        ]:
            eng.dma_start(out=xs[t][:, cols[0]:cols[1]],
                          in_=x_d[t * 128:(t + 1) * 128, cols[0]:cols[1]])

        # ---- constant loads on the gpsimd queue (25ns triggers) ----
        def load4(dram, shape, dt, tagp):
            ts = []
            for t in range(4):
                s = const.tile(shape, dt, tag=f"{tagp}{t}")
                nc.gpsimd.dma_start(out=s, in_=dram[t * 128:(t + 1) * 128, :])
                ts.append(s)
            return ts

        gmap = load4(gmap_d, [128, G], F32, "gmap")
        gmapt = const.tile([G, C], F32, tag="gmapt")
        nc.gpsimd.dma_start(out=gmapt, in_=gmapt_d[:, :])
        gam = load4(gam_d, [128, 1], F32, "gam")
        bet = load4(bet_d, [128, 1], F32, "bet")
        qwt = load4(qwt_d, [128, C], BF16, "qwt")
        kwt = load4(kwt_d, [128, C], BF16, "kwt")
        vwt = load4(vwt_d, [128, C], BF16, "vwt")
        qb = load4(qb_d, [128, 1], F32, "qb")
        pwt = load4(pwt_d, [128, C], BF16, "pwt")
        pb2 = load4(pb2_d, [128, 1], F32, "pb2")
        eps32 = const.tile([G, 1], F32, tag="eps32")
        nc.vector.memset(eps32, EPS)

        # ---- persistent tiles ----
        hs = [data.tile([128, N], BF16, tag=f"h{t}", name=f"h{t}") for t in range(4)]
        qts = [data.tile([128, NQ], BF16, tag=f"q{ct}", name=f"q{ct}") for ct in range(4)]
        kts = [data.tile([128, N], BF16, tag=f"k{ct}", name=f"k{ct}") for ct in range(4)]
        vts = [data.tile([128, H * 65], BF16, tag=f"vt{mt}", name=f"vt{mt}") for mt in range(16)]
        attns = [data.tile([128, NQ], BF16, tag=f"attn{c}", name=f"attn{c}") for c in range(4)]
        ppart = {(mo, n): data.tile([128, 512], F32, tag=f"pp{mo}_{n}", name=f"pp{mo}_{n}")
                 for mo in range(4) for n in range(2)}

        for mt in range(16):
            (nc.vector if DBG_NO_GPSMEMSET else nc.gpsimd).memset(vts[mt], 1.0)

        # ---- groupnorm: per-x-tile stats + chain (pipelined) ----
        def emit_stats(t):
            st = work.tile([128, 4, 6], F32, tag="bnst", name=f"bnst{t}")
            for sg in range(4):
                nc.vector.bn_stats(out=st[:, sg, :],
                                   in_=xs[t][:, sg * 512:(sg + 1) * 512])
            mv = work.tile([128, 2], F32, tag="bnmv", name=f"bnmv{t}")
            nc.vector.bn_aggr(out=mv, in_=st)
            s2 = work.tile([128, 2], F32, tag="s2", name=f"s2_{t}")
            nc.vector.tensor_copy(out=s2[:, 0:1], in_=mv[:, 0:1])
            nc.vector.tensor_mul(out=s2[:, 1:2], in0=mv[:, 0:1], in1=mv[:, 0:1])
            nc.vector.tensor_add(out=s2[:, 1:2], in0=s2[:, 1:2], in1=mv[:, 1:2])
            return s2

        def emit_chain(t, s2):
            gps = psGen.tile([128, 512], F32, tag="gen", name=f"gps{t}")
            nc.tensor.matmul(out=gps[0:G, 0:2], lhsT=gmap[t], rhs=s2,
                             start=True, stop=True)
            mvg = work.tile([G, 2], F32, tag="mvg", name=f"mvg{t}")
            nc.vector.tensor_scalar(out=mvg, in0=gps[0:G, 0:2], scalar1=1.0 / 16,
                                    scalar2=None, op0=ALU.mult)
            varg = work.tile([G, 1], F32, tag="varg", name=f"varg{t}")
            nc.vector.tensor_mul(out=varg, in0=mvg[:, 0:1], in1=mvg[:, 0:1])
            nc.vector.tensor_tensor(out=varg, in0=mvg[:, 1:2], in1=varg,
                                    op=ALU.subtract)
            sd = work.tile([G, 1], F32, tag="sd", name=f"sd{t}")
            nc.scalar.activation(out=sd, in_=varg, func=AF.Sqrt, bias=eps32)
            rsg = work.tile([G, 1], F32, tag="rsg", name=f"rsg{t}")
            nc.vector.reciprocal(out=rsg, in_=sd)
            gvals = work.tile([G, 2], F32, tag="gvals", name=f"gvals{t}")
            nc.vector.tensor_copy(out=gvals[:, 0:1], in_=rsg)
            nc.vector.tensor_copy(out=gvals[:, 1:2], in_=mvg[:, 0:1])
            bc = psGen.tile([128, 512], F32, tag="gen", name=f"bcm{t}")
            nc.tensor.matmul(out=bc[:, 0:2], lhsT=gmapt[:, t * 128:(t + 1) * 128],
                             rhs=gvals, start=True, stop=True)
            a_t = work.tile([128, 1], F32, tag="a_t", name=f"a{t}")
            nc.vector.tensor_mul(out=a_t, in0=bc[:, 0:1], in1=gam[t])
            b_t = work.tile([128, 1], F32, tag="b_t", name=f"b{t}")
            nc.vector.tensor_mul(out=b_t, in0=bc[:, 1:2], in1=a_t)
            nc.vector.tensor_tensor(out=b_t, in0=bet[t], in1=b_t, op=ALU.subtract)
            # apply: DVE does cols 0:1024, gpsimd does 1024:2048
            nc.vector.tensor_scalar(out=hs[t][:, 0:1024], in0=xs[t][:, 0:1024],
                                    scalar1=a_t, scalar2=b_t,
                                    op0=ALU.mult, op1=ALU.add)
            (nc.vector if DBG_NO_GPSH else nc.gpsimd).tensor_scalar(
                out=hs[t][:, 1024:2048], in0=xs[t][:, 1024:2048],
                scalar1=a_t, scalar2=b_t, op0=ALU.mult, op1=ALU.add)

        s2_0 = emit_stats(0)
        s2_1 = emit_stats(1)
        emit_chain(0, s2_0)
        s2_2 = emit_stats(2)
        emit_chain(1, s2_1)
        s2_3 = emit_stats(3)
        emit_chain(2, s2_2)
        emit_chain(3, s2_3)

        # dummy exp to pull the ACT exp-table load off the critical path
        dummy = work.tile([G, 1], F32, tag="dummy")
        nc.scalar.activation(out=dummy, in_=eps32, func=AF.Exp)

        # ---- filler tasks (run on PE between score/attnV pairs) ----
        def q_group(ct, n):
            def go():
                ps = psGen.tile([128, 512], F32, tag="gen", name=f"qps{ct}_{n}")
                for kt in range(4):
                    nc.tensor.matmul(out=ps, lhsT=qwt[kt][:, ct * 128:(ct + 1) * 128],
                                     rhs=hs[kt][:, n * 512:(n + 1) * 512],
                                     start=(kt == 0), stop=(kt == 3))
                nc.vector.tensor_scalar(out=qts[ct][:, n * 512:(n + 1) * 512],
                                        in0=ps, scalar1=qb[ct], scalar2=None,
                                        op0=ALU.add)
            return go

        def k_group(ct, j):
            def go():
                ps = psGen.tile([128, 512], F32, tag="gen", name=f"kps{ct}_{j}")
                for kt in range(4):
                    nc.tensor.matmul(out=ps, lhsT=kwt[kt][:, ct * 128:(ct + 1) * 128],
                                     rhs=hs[kt][:, j * 512:(j + 1) * 512],
                                     start=(kt == 0), stop=(kt == 3))
                nc.vector.tensor_copy(out=kts[ct][:, j * 512:(j + 1) * 512], in_=ps)
            return go

        def vt_group(mt):
            def go():
                ps = psGen.tile([128, 512], F32, tag="gen", name=f"vtps{mt}")
                for kt in range(4):
                    nc.tensor.matmul(out=ps, lhsT=hs[kt][:, mt * 128:(mt + 1) * 128],
                                     rhs=vwt[kt][:, 0:512],
                                     start=(kt == 0), stop=(kt == 3))
                nc.vector.tensor_copy(
                    out=vts[mt].rearrange("p (h w) -> p h w", h=H)[:, :, 0:HC],
                    in_=ps.rearrange("p (h w) -> p h w", h=H),
                )
            return go

        def projA_group(mo, n):
            # head pairs 0,1 -> SBUF partial
            def go():
                if DBG_STAGE:
                    return
                pps = psGen.tile([128, 512], F32, tag="gen", name=f"ppsA{mo}_{n}")
                for c in range(2):
                    nc.tensor.matmul(out=pps,
                                     lhsT=pwt[c][:, mo * 128:(mo + 1) * 128],
                                     rhs=attns[c][:, n * 512:(n + 1) * 512],
                                     start=(c == 0), stop=(c == 1))
                nc.vector.tensor_copy(out=ppart[(mo, n)], in_=pps)
            return go

        def projB_group(mo, n):
            # head pairs 2,3 + partial + bias + residual -> out DMA
            def go():
                if DBG_STAGE:
                    return
                pps = psGen.tile([128, 512], F32, tag="gen", name=f"ppsB{mo}_{n}")
                for c in range(2, 4):
                    nc.tensor.matmul(out=pps,
                                     lhsT=pwt[c][:, mo * 128:(mo + 1) * 128],
                                     rhs=attns[c][:, n * 512:(n + 1) * 512],
                                     start=(c == 2), stop=(c == 3))
                osb = osbp.tile([128, 512], F32, tag="osb", name=f"osb{mo}_{n}")
                nc.vector.scalar_tensor_tensor(
                    out=osb, in0=pps, scalar=pb2[mo], in1=ppart[(mo, n)],
                    op0=ALU.add, op1=ALU.add)
                nc.vector.tensor_add(out=osb, in0=osb,
                                     in1=xs[mo][:, n * 512:(n + 1) * 512])
                nc.sync.dma_start(
                    out=out_d[mo * 128:(mo + 1) * 128, n * 512:(n + 1) * 512],
                    in_=osb)
            return go

        import collections
        fillers = collections.deque()
        emitted = set()

        def push(fn, key=None):
            fillers.append((fn, key))

        def pop_filler():
            if fillers:
                fn, key = fillers.popleft()
                fn()
                if key is not None:
                    emitted.add(key)

        def ensure(key):
            while key not in emitted:
                assert fillers, f"filler queue empty but {key} not emitted"
                pop_filler()

        # prologue: q(ct0,n0) + k(ct0,j0) gate the first scores
        q_group(0, 0)()
        emitted.add(("q", 0, 0))
        k_group(0, 0)()
        emitted.add(("k", 0, 0))

        # deadline-ordered: vt(mt) needed at slot mt+2, k(0,j) at slot 4j,
        # q(0,1) at slot 16
        push(vt_group(0), ("vt", 0))
        push(vt_group(1), ("vt", 1))
        push(vt_group(2), ("vt", 2))
        push(vt_group(3), ("vt", 3))
        push(k_group(0, 1), ("k", 0, 1))
        for mt in range(4, 8):
            push(vt_group(mt), ("vt", mt))
        push(k_group(0, 2), ("k", 0, 2))
        for mt in range(8, 12):
            push(vt_group(mt), ("vt", mt))
        push(k_group(0, 3), ("k", 0, 3))
        push(q_group(0, 1), ("q", 0, 1))
        for mt in range(12, 16):
            push(vt_group(mt), ("vt", mt))

        # ---- attention slot loop ----
        slots = [(ct, n, mt) for ct in range(4) for n in range(2) for mt in range(16)]
        pending_avs = collections.deque()  # lag-2 attnV pipeline
        avE = {}
        avO = {}

        def emit_scores(ct, n, mt):
            ensure(("q", ct, n))
            ensure(("k", ct, mt // 4))
            sc = psSC.tile([128, NQ], F32, tag="sc", name=f"sc_{ct}_{n}_{mt}")
            for hp in range(2):
                hb = hp * 64
                nc.tensor.matmul(
                    out=sc[:, hp * 512:(hp + 1) * 512],
                    lhsT=kts[ct][hb:hb + 64, mt * 128:(mt + 1) * 128],
                    rhs=qts[ct][hb:hb + 64, n * 512:(n + 1) * 512],
                    start=True, stop=True)
            et = etp.tile([128, NQ], BF16, tag="exp", name=f"et_{ct}_{n}_{mt}")
            nc.scalar.activation(out=et, in_=sc, func=AF.Exp, scale=float(SCALE))
            return et

        def emit_av(ct, n, mt, et):
            ensure(("vt", mt))
            if mt == 0:
                avE[(ct, n)] = psAV.tile([128, 512], F32, tag="av",
                                         name=f"avE{ct}_{n}")
                avO[(ct, n)] = psAV.tile([128, 512], F32, tag="av",
                                         name=f"avO{ct}_{n}")
            for hp in range(2):
                h = 2 * ct + hp
                dst = avE[(ct, n)] if hp == 0 else avO[(ct, n)]
                nc.tensor.matmul(
                    out=dst[0:65, :],
                    lhsT=vts[mt][:, 65 * h:65 * h + 65],
                    rhs=et[:, hp * 512:(hp + 1) * 512],
                    start=(mt == 0), stop=(mt == 15))

        def emit_drain(ct, n):
            if DBG_STAGE and DBG_STAGE < 4:
                return
            # reciprocal of both heads' denominators (PSUM row 64, stays on
            # partition 64 — DVE lanes cannot shift partitions), then a DRAM
            # bounce to broadcast the [1,1024] row across 64 partitions.
            psE, psO = avE[(ct, n)], avO[(ct, n)]
            rec = recp.tile([65, NQ], F32, tag="rec", name=f"rec{ct}_{n}")
            nc.vector.reciprocal(out=rec[64:65, 0:512], in_=psE[64:65, 0:512])
            nc.vector.reciprocal(out=rec[64:65, 512:1024], in_=psO[64:65, 0:512])
            bcast = recp.tile([64, NQ], F32, tag="bc", name=f"bcr{ct}_{n}")
            if DBG_NO_BOUNCE:
                nc.vector.memset(bcast, 1.0)
            else:
                recd = dpool.tile([1, NQ], F32, tag="recd", name=f"recd{ct}_{n}")
                for hh in range(2):
                    nc.gpsimd.dma_start(out=recd[0:1, hh * 512:(hh + 1) * 512],
                                        in_=rec[64:65, hh * 512:(hh + 1) * 512])
                for hh in range(2):
                    nc.gpsimd.dma_start(
                        out=bcast[:, hh * 512:(hh + 1) * 512],
                        in_=recd[0:1, hh * 512:(hh + 1) * 512]
                        .to_broadcast([64, 512]))
            nc.vector.tensor_mul(
                out=attns[ct][0:64, n * 512:(n + 1) * 512],
                in0=psE[0:64, 0:512], in1=bcast[:, 0:512])
            tmp = tmpp.tile([64, 512], BF16, tag="tmp", name=f"tmp{ct}_{n}")
            nc.vector.tensor_mul(out=tmp, in0=psO[0:64, 0:512],
                                 in1=bcast[:, 512:1024])
            if not DBG_NO_SHIFT:
                nc.gpsimd.dma_start(
                    out=attns[ct][64:128, n * 512:(n + 1) * 512], in_=tmp)

        for i, (ct, n, mt) in enumerate(slots):
            # inject follow-on filler tasks at pass starts
            if mt == 0:
                if n == 1 and ct < 3:
                    for j in range(2):
                        push(q_group(ct + 1, j), ("q", ct + 1, j))
                    for j in range(4):
                        push(k_group(ct + 1, j), ("k", ct + 1, j))
                if ct == 2 and n == 0:
                    for mo in range(4):
                        push(projA_group(mo, 0))
                if ct == 2 and n == 1:
                    for mo in range(4):
                        push(projA_group(mo, 1))
            if ct == 3 and n == 1 and mt == 4:
                for mo in range(4):
                    push(projB_group(mo, 0))

            if DBG_STAGE == 1:
                if i % 2 == 1:
                    pop_filler()
                continue
            et = emit_scores(ct, n, mt)
            if DBG_STAGE == 2:
                if i % 2 == 1:
                    pop_filler()
                continue

            if len(pending_avs) >= 4:
                pct, pn, pmt, pet = pending_avs.popleft()
                emit_av(pct, pn, pmt, pet)
                if pmt == 15:
                    emit_drain(pct, pn)
            pending_avs.append((ct, n, mt, et))
            if i % 2 == 1:
                pop_filler()

        while pending_avs:
            pct, pn, pmt, pet = pending_avs.popleft()
            emit_av(pct, pn, pmt, pet)
            if pmt == 15:
                emit_drain(pct, pn)

        for mo in range(4):
            push(projB_group(mo, 1))
        while fillers:
            pop_filler()

        if DBG_STAGE:
            # dummy output so the kernel still produces 'out'
            for mo in range(4):
                for n in range(2):
                    osb = osbp.tile([128, 512], F32, tag="osb", name=f"dosb{mo}_{n}")
                    nc.vector.memset(osb, 0.0)
                    nc.sync.dma_start(
                        out=out_d[mo * 128:(mo + 1) * 128, n * 512:(n + 1) * 512],
                        in_=osb)

    nc.compile()
    return nc


_NC_CACHE = None


def _get_nc():
    global _NC_CACHE
    if _NC_CACHE is None:
        _NC_CACHE = build_bacc()
    return _NC_CACHE


def kernel(x, gn_gamma, gn_beta, q_w, q_b, k_w, k_b, v_w, v_b, proj_w, proj_b):
    global LAST_RESULT
    x = np.asarray(x, np.float32)
    B = x.shape[0]
    bf = ml_dtypes.bfloat16

    gmap = np.zeros((C, G), np.float32)
    gmap[np.arange(C), np.arange(C) // 16] = 1.0

    pb2 = (np.asarray(proj_b, np.float32)
           + np.asarray(proj_w, np.float32) @ np.asarray(v_b, np.float32))

    shared = {
        "qwt": np.ascontiguousarray(np.asarray(q_w, np.float32).T.astype(bf)),
        "kwt": np.ascontiguousarray(np.asarray(k_w, np.float32).T.astype(bf)),
        "vwt": np.ascontiguousarray(np.asarray(v_w, np.float32).T.astype(bf)),
        "pwt": np.ascontiguousarray(np.asarray(proj_w, np.float32).T.astype(bf)),
        "qb": np.asarray(q_b, np.float32).reshape(C, 1),
        "pb2": pb2.reshape(C, 1),
        "gam": np.asarray(gn_gamma, np.float32).reshape(C, 1),
        "bet": np.asarray(gn_beta, np.float32).reshape(C, 1),
        "gmap": gmap,
        "gmapt": np.ascontiguousarray(gmap.T),
    }

    in_maps = []
    for i in range(8):
        b, half = i // 2, i % 2
        xb = np.ascontiguousarray(np.roll(x[b], -half * NQ, axis=1))
        in_maps.append({"x": xb, **shared})

    nc = _get_nc()
    res = run_bass_kernel_spmd(nc, in_maps, core_ids=list(range(8)), trace=TRACE)
    LAST_RESULT = res

    out = np.empty((B, C, N), np.float32)
    for i in range(8):
        b, half = i // 2, i % 2
        out[b][:, half * NQ:(half + 1) * NQ] = res.results[i]["out"]
    return out
